# revision 27
# baseline (speedup 1.0000x reference)
"""Multi-head GAT layer (4 heads, mean-aggregated) + residual + GraphNorm + gelu
on 8 Trainium2 NeuronCores (SPMD, one NEFF on all cores).

Strategy (v2 — gather-descriptor-rate oriented):
  - dst nodes partitioned contiguously across 8 cores (12500 each); every edge
    is processed by the core owning its destination.
  - Every core builds a record table for ALL nodes in fp8: 512B records
    [xl interleaved (c*4+h) fp8 x256 | 1.0 fp8 x4 | u=e^asrc bf16 x4 |
     v=e^(0.2 asrc) bf16 x4 | junk]. Table rows are partition-major per chunk
    so phase-1 writes are contiguous per partition (few large descriptors).
  - Softmax trick: exp(leakyrelu(s+d)) = e^d * max(e^s, e^(0.2s) e^(-0.8d)).
    The per-dst factor e^d cancels in the softmax, so the edge weight is
    w = max(u_src, v_src * r_dst) with r = e^(-0.8 adst) — no per-edge
    exp/prelu needed; r is gathered per-edge via a tiny one-hot matmul (m2).
  - Edge phase: per PAIR of dst tiles, one merged dma_gather per chunk
    (~1k indices per call — SWDGE fixed overhead amortized). Self-loops are
    NOT gathered: their records are read with one contiguous DMA per tile
    from a per-core own-record table, and appear as a final "self block"
    whose one-hots are identity.
  - Aggregation: per 128-edge block, rhs = w (x) g[0:260] (the 4 trailing
    ones-columns turn into the softmax denominators), one-hot scatter matmul
    (m1 fp8 lhsT x bf16 rhs) accumulating [128 dst, 260] in PSUM.
  - GraphNorm: per-graph sums via one-hot matmuls, AllReduce, affine + gelu.

All structure (block layout, one-hots, gather indices) is host-computed from
edge_index/batch only and padded to the max over cores so one SPMD NEFF
serves all 8 cores.
"""
import numpy as np
import ml_dtypes

_BF16 = ml_dtypes.bfloat16
_FP8 = ml_dtypes.float8_e4m3
_F32 = np.float32

N, F, C, H, E, B = 100000, 128, 64, 4, 1200000, 8
NCORE = 8
NOWN = N // NCORE              # 12500
TILES = NOWN // 128 + 1        # 98 (last tile 84 rows)
LAST_ROWS = NOWN - (TILES - 1) * 128   # 84
NCH = 4
CHR = 25088                    # chunk rows (196 tiles of 128)
NTC = CHR // 128               # 196
NPAD = NCH * CHR               # 100352
REC = 512                      # record bytes (fp8 elems)
RECH = REC // 2                # bf16 view cols
G1 = 7                         # sweep group tiles (196 % 7 == 0, 98 % 7 == 0)
NPAIR = (TILES + 1) // 2       # 49
CALL_LIMIT = 1280              # max idx per dma_gather call


def _cdiv(a, b):
    return (a + b - 1) // b


def _host_prep(x, edge_index, batch, W, att_src, att_dst, bias_gat,
               res_W, res_b, gn_weight, gn_bias, gn_mean_scale):
    x = np.asarray(x, _F32)
    W = np.asarray(W, _F32)
    att_src = np.asarray(att_src, _F32)
    att_dst = np.asarray(att_dst, _F32)
    res_W = np.asarray(res_W, _F32)
    batch = np.asarray(batch).astype(np.int64)

    # fused right matrix [F, 328] = [W.T interleaved | As.T | Ad.T | res_W.T]
    W3 = W.reshape(H, C, F)
    As = (att_src[:, :, None] * W3).sum(1)          # [H, F]
    Ad = (att_dst[:, :, None] * W3).sum(1)          # [H, F]
    WT = W.T                                        # [F, 256] col h*64+c
    perm = np.empty(H * C, np.int64)                # newcol c*4+h <- old h*64+c
    cc, hh = np.meshgrid(np.arange(C), np.arange(H), indexing="ij")
    perm[(cc * H + hh).ravel()] = (hh * C + cc).ravel()
    WTi = WT[:, perm]                               # [F, 256] interleaved
    Rfull = np.concatenate([WTi, As.T, Ad.T, res_W.T], axis=1).astype(_BF16)

    xT = np.zeros((F, NPAD), _BF16)
    xT[:, :N] = x.T.astype(_BF16)

    # ---- edge structure (NO self loops in the gather stream) ----
    src = np.asarray(edge_index[0]).astype(np.int64)
    dst = np.asarray(edge_index[1]).astype(np.int64)
    owner = dst // NOWN
    tl = (dst % NOWN) // 128
    dl = (dst % NOWN) % 128
    ch = src // CHR
    nl = src % CHR
    idxval = ((nl % 128) * NTC + nl // 128).astype(np.int16)  # chunk-local row

    key = (owner * TILES + tl) * NCH + ch
    order = np.lexsort((src, key))
    s_key = key[order]
    s_dl = dl[order]
    s_idx = idxval[order]

    counts = np.bincount(key, minlength=NCORE * TILES * NCH).reshape(
        NCORE, TILES, NCH)
    K_tc = counts.max(axis=0)                       # [TILES, NCH]
    nb_tc = _cdiv(K_tc, 128)                        # blocks per (tile, chunk)
    assert (nb_tc >= 1).all()
    ns_t = nb_tc.sum(axis=1)                        # real blocks per tile
    ns1_t = ns_t + 1                                # + self block
    m1off = np.zeros(TILES + 1, np.int64)
    m1off[1:] = np.cumsum(ns1_t)
    NSUB1 = int(m1off[-1])
    MAXNS1 = int(ns1_t.max())

    gstart = np.zeros(NCORE * TILES * NCH + 1, np.int64)
    gstart[1:] = np.cumsum(counts.reshape(-1))

    # ---- pair regions & gather calls ----
    # region blocks: [c0: t0-blocks, t1-blocks][c1: ...]...[self t0][self t1]
    segoff = np.zeros((NPAIR, NCH), np.int64)
    RB = np.zeros(NPAIR, np.int64)
    pair_tiles = []
    for p in range(NPAIR):
        ts = [2 * p] + ([2 * p + 1] if 2 * p + 1 < TILES else [])
        pair_tiles.append(ts)
        off = 0
        for c in range(NCH):
            segoff[p, c] = off
            off += int(sum(nb_tc[t, c] for t in ts))
        RB[p] = off + len(ts)                       # + self blocks
    MAXRB = int(RB.max())
    selfoff = {}
    for p in range(NPAIR):
        base = int(RB[p]) - len(pair_tiles[p])
        for qi, t in enumerate(pair_tiles[p]):
            selfoff[t] = base + qi

    # per-tile segments: (j0 tile-block idx, k blocks, b0 region block)
    segs_t = []
    blocks_t = []
    for t in range(TILES):
        p, q = t // 2, t % 2
        segs = []
        j0 = 0
        for c in range(NCH):
            k = int(nb_tc[t, c])
            b0 = int(segoff[p, c])
            if q == 1:
                b0 += int(nb_tc[2 * p, c])
            segs.append((j0, k, b0))
            j0 += k
        segs.append((j0, 1, selfoff[t]))            # self block
        segs_t.append(segs)
        rbl = []
        for (j0, k, b0) in segs:
            rbl.extend(range(b0, b0 + k))
        blocks_t.append(rbl)                        # region block per tile blk

    # gather calls: per (pair, chunk) merged unless over CALL_LIMIT
    # call = (c, col_off, L, out_b0, [(t, slots_t), ...])
    calls_p = []
    oc = 0
    for p in range(NPAIR):
        calls = []
        for c in range(NCH):
            parts = [(t, int(nb_tc[t, c]) * 128) for t in pair_tiles[p]]
            L = sum(s for _, s in parts)
            if L <= CALL_LIMIT:
                calls.append((c, oc, L, int(segoff[p, c]), parts))
                oc += L // 16
            else:
                b0 = int(segoff[p, c])
                for t, s in parts:
                    calls.append((c, oc, s, b0, [(t, s)]))
                    oc += s // 16
                    b0 += s // 128
        calls_p.append(calls)
    IDXC = oc

    # ---- per-core tensors ----
    # slot bookkeeping for idx16 fill: flat col position of (t, c) stream
    stream_base = {}
    for p in range(NPAIR):
        for (c, coff, L, out_b0, parts) in calls_p[p]:
            pos = coff * 16
            for t, s in parts:
                stream_base[(t, c)] = pos
                pos += s

    onesrow = np.ones(128, _F32)
    in_maps = []
    for k in range(NCORE):
        idxflat = np.zeros(IDXC * 16, np.int16)
        m1 = np.zeros((128, NSUB1, 128), _FP8)
        m2 = np.zeros((128, NSUB1, 128), _FP8)
        for t in range(TILES):
            # self block identity
            v = 128 if t < TILES - 1 else LAST_ROWS
            jself = m1off[t] + ns_t[t]
            lanes = np.arange(v)
            m1[lanes, jself, lanes] = 1.0
            m2[lanes, jself, lanes] = 1.0
            cnb = 0
            for c in range(NCH):
                n = int(counts[k, t, c])
                a = int(gstart[(k * TILES + t) * NCH + c])
                if n:
                    sl = np.arange(n)
                    blk = m1off[t] + cnb + sl // 128
                    lane = sl % 128
                    dlv = s_dl[a:a + n]
                    m1[lane, blk, dlv] = 1.0
                    m2[dlv, blk, lane] = 1.0
                    base = stream_base[(t, c)]
                    idxflat[base:base + n] = s_idx[a:a + n]
                cnb += int(nb_tc[t, c])
        idx16 = np.ascontiguousarray(
            np.tile(idxflat.reshape(-1, 16).T, (8, 1)))     # [128, IDXC]
        m1c = np.ascontiguousarray(m1.reshape(128, NSUB1 * 128))
        m2c = np.ascontiguousarray(m2.reshape(128, NSUB1 * 128))

        base = k * NOWN
        xTo = np.zeros((F, TILES * 128), _BF16)
        xTo[:, :NOWN] = x[base:base + NOWN].T.astype(_BF16)
        bslice = batch[base:base + NOWN]
        node_ids = np.arange(NOWN)
        pp = node_ids % 128
        tt = node_ids // 128
        onehot_b = np.zeros((128, TILES * 8), _BF16)
        onehot_b[pp, tt * 8 + bslice] = 1.0
        onehotT = np.zeros((8, TILES * 128), _BF16)
        onehotT[bslice, node_ids] = 1.0

        in_maps.append({
            "xT": xT, "xTo": xTo, "Rfull": Rfull, "idx16": idx16,
            "m1all": m1c, "m2all": m2c, "onehot_b": onehot_b,
            "onehotT": onehotT,
        })

    bc_row = np.tile((np.asarray(bias_gat, _F32)
                      + np.asarray(res_b, _F32))[None, :], (128, 1))
    ident = np.eye(128, dtype=_BF16)
    gms = np.asarray(gn_mean_scale, _F32)
    cnt = np.bincount(batch, minlength=B).astype(_F32)
    gn_pack = np.zeros((8, 4 * C + 2), _F32)
    gn_pack[:, 0:C] = np.asarray(gn_weight, _F32)[None, :]
    gn_pack[:, C:2 * C] = np.asarray(gn_bias, _F32)[None, :]
    gn_pack[:, 2 * C:3 * C] = gms[None, :]
    gn_pack[:, 3 * C:4 * C] = (gms * (2.0 - gms))[None, :]
    gn_pack[:, 4 * C] = 1.0 / cnt
    gn_pack[:, 4 * C + 1] = 1e-5
    for m in in_maps:
        m.update({"bc_row": bc_row, "gn_pack": gn_pack, "ident": ident})

    cfg = {
        "nb_tc": nb_tc, "ns_t": ns_t, "ns1_t": ns1_t, "m1off": m1off,
        "NSUB1": NSUB1, "MAXNS1": MAXNS1, "MAXRB": MAXRB, "RB": RB,
        "segs_t": segs_t, "blocks_t": blocks_t, "calls_p": calls_p,
        "IDXC": IDXC, "selfoff": selfoff, "pair_tiles": pair_tiles,
    }
    return cfg, in_maps


def _build_nc(cfg):
    import concourse.bacc as bacc
    import concourse.mybir as mybir
    import concourse.tile as tile

    AF = mybir.ActivationFunctionType
    OP = mybir.AluOpType
    f32 = mybir.dt.float32
    bf16 = mybir.dt.bfloat16
    fp8 = mybir.dt.float8e4
    i16 = mybir.dt.int16

    ns_t, ns1_t, m1off = cfg["ns_t"], cfg["ns1_t"], cfg["m1off"]
    NSUB1, MAXNS1, MAXRB = cfg["NSUB1"], cfg["MAXNS1"], cfg["MAXRB"]
    segs_t, blocks_t, calls_p = cfg["segs_t"], cfg["blocks_t"], cfg["calls_p"]
    IDXC, selfoff, pair_tiles = cfg["IDXC"], cfg["selfoff"], cfg["pair_tiles"]

    nc = bacc.Bacc("TRN2", target_bir_lowering=False, num_swdge_queues=4,
                   dynamic_dma_scratch_size=49152)

    xT = nc.declare_dram_parameter("xT", [F, NPAD], bf16, isOutput=False)
    xTo = nc.declare_dram_parameter("xTo", [F, TILES * 128], bf16, isOutput=False)
    Rfull = nc.declare_dram_parameter("Rfull", [F, 328], bf16, isOutput=False)
    idx16 = nc.declare_dram_parameter("idx16", [128, IDXC], i16, isOutput=False)
    m1all = nc.declare_dram_parameter("m1all", [128, NSUB1 * 128], fp8, isOutput=False)
    m2all = nc.declare_dram_parameter("m2all", [128, NSUB1 * 128], fp8, isOutput=False)
    onehot_b = nc.declare_dram_parameter("onehot_b", [128, TILES * 8], bf16, isOutput=False)
    onehotT = nc.declare_dram_parameter("onehotT", [8, TILES * 128], bf16, isOutput=False)
    bc_row = nc.declare_dram_parameter("bc_row", [128, C], f32, isOutput=False)
    gn_pack = nc.declare_dram_parameter("gn_pack", [8, 4 * C + 2], f32, isOutput=False)
    ident = nc.declare_dram_parameter("ident", [128, 128], bf16, isOutput=False)
    out = nc.declare_dram_parameter("out", [NOWN, C], f32, isOutput=True)

    tables = [nc.dram_tensor(f"table{c}", [CHR, REC], fp8) for c in range(NCH)]
    own_rec = nc.dram_tensor("own_rec", [TILES * 128, REC], fp8)
    cc_in = nc.dram_tensor("cc_in", [8, 2 * C], f32)
    cc_out = nc.dram_tensor("cc_out", [8, 2 * C], f32)

    qn = [0]

    def next_q():
        q = qn[0]
        qn[0] = (q + 1) % 4
        return q

    with tile.TileContext(nc) as tc:
        with (
            tc.tile_pool(name="const", bufs=1) as cp,
            tc.tile_pool(name="persist", bufs=1) as pers,
            tc.tile_pool(name="xload", bufs=2) as xp,
            tc.tile_pool(name="orec", bufs=2) as orp,
            tc.tile_pool(name="rec", bufs=2) as rp,
            tc.tile_pool(name="gat", bufs=3) as gp,
            tc.tile_pool(name="m1", bufs=3) as m1p,
            tc.tile_pool(name="m2", bufs=3) as m2p,
            tc.tile_pool(name="rhs", bufs=8) as rhp,
            tc.tile_pool(name="small", bufs=8) as smp,
            tc.tile_pool(name="idxp", bufs=3) as ixp,
            tc.tile_pool(name="ohtp", bufs=4) as ohp,
            tc.tile_pool(name="outp", bufs=4) as otp,
        ):
            rf_sb = cp.tile([F, 328], bf16)
            nc.sync.dma_start(rf_sb[:], Rfull[:])
            bc_sb = cp.tile([128, C], f32)
            nc.sync.dma_start(bc_sb[:], bc_row[:])
            ohb_sb = cp.tile([128, TILES * 8], bf16)
            nc.sync.dma_start(ohb_sb[:], onehot_b[:])
            gn_sb = cp.tile([8, 4 * C + 2], f32)
            nc.sync.dma_start(gn_sb[:], gn_pack[:])
            id_sb = cp.tile([128, 128], bf16)
            nc.sync.dma_start(id_sb[:], ident[:])

            resid_sb = pers.tile([128, TILES * C], bf16)
            h_sb = pers.tile([128, TILES * C], bf16)
            r_sb = pers.tile([128, TILES * 4], fp8)
            stats_sb = pers.tile([8, 2 * C], f32)
            nc.vector.memset(stats_sb[:], 0.0)

            with (
                tc.tile_pool(name="pso", bufs=1, space="PSUM") as pso,
                tc.tile_pool(name="psb", bufs=3, space="PSUM") as psb,
                tc.tile_pool(name="pagg", bufs=2, space="PSUM") as pagg,
                tc.tile_pool(name="pape", bufs=1, space="PSUM") as pape,
                tc.tile_pool(name="pst", bufs=1, space="PSUM") as pst,
            ):
                def sweep_group(pool, psum_pool, src_t, col0, ncols, dest, g,
                                own, prime):
                    """One group of G1 node tiles: matmuls + casts + write."""
                    xs = xp.tile([F, G1 * 128], bf16, tag="xo" if own else "x")
                    nc.sync.dma_start(xs[:], src_t[:, col0:col0 + G1 * 128])
                    rec = pool.tile([128, G1 * REC], fp8,
                                    tag="or" if own else "rec")
                    if prime:
                        nc.vector.memset(rec[:], 1.0)
                    rh = rec.bitcast(bf16)
                    for i in range(G1):
                        ps = psum_pool.tile([128, ncols], f32,
                                            tag="pso" if own else "psb")
                        nc.tensor.matmul(ps[:], lhsT=xs[:, i * 128:(i + 1) * 128],
                                         rhs=rf_sb[:, 0:ncols],
                                         start=True, stop=True)
                        if i % 2 == 0:
                            nc.scalar.copy(rec[:, i * REC:i * REC + 256],
                                           ps[:, 0:256])
                            nc.vector.tensor_copy(
                                rh[:, i * RECH + 130:i * RECH + 134],
                                ps[:, 256:260])
                        else:
                            nc.vector.tensor_copy(rec[:, i * REC:i * REC + 256],
                                                  ps[:, 0:256])
                            nc.scalar.copy(rh[:, i * RECH + 130:i * RECH + 134],
                                           ps[:, 256:260])
                        if own:
                            t = g * G1 + i
                            nc.scalar.activation(out=r_sb[:, t * 4:(t + 1) * 4],
                                                 in_=ps[:, 260:264], func=AF.Exp,
                                                 scale=-0.8)
                            nc.vector.tensor_tensor(
                                out=resid_sb[:, t * C:(t + 1) * C],
                                in0=ps[:, 264:328], in1=bc_sb[:], op=OP.add)
                    nc.sync.dma_start(
                        dest.rearrange("(p t) e -> p t e", p=128)
                        [:, g * G1:(g + 1) * G1, :],
                        rec[:].rearrange("p (i e) -> p i e", e=REC))

                # phase 1b: own-node sweep -> own_rec + r + resid
                for g in range(TILES // G1):
                    sweep_group(orp, pso, xTo, g * G1 * 128, 328, own_rec, g,
                                own=True, prime=(g < 2))
                # phase 1a: full-node sweep -> chunk tables
                for c in range(NCH):
                    for g in range(NTC // G1):
                        sweep_group(rp, psb, xT, c * CHR + g * G1 * 128, 260,
                                    tables[c], g, own=False,
                                    prime=(c == 0 and g < 2))

                # ---- phase 2: edge sweep, software-pipelined ----
                gprs = {}

                def emit_gathers(p):
                    gpr = gp.tile([128, MAXRB * REC], fp8, tag="g")
                    if p < 3:
                        nc.gpsimd.memset(gpr[:], 0.0)
                    ncols = sum(cl[2] // 16 for cl in calls_p[p])
                    c0 = calls_p[p][0][1]
                    ix = ixp.tile([128, ncols], i16, tag="ix")
                    nc.sync.dma_start(ix[:], idx16[:, c0:c0 + ncols])
                    for (c, coff, L, out_b0, parts) in calls_p[p]:
                        nc.gpsimd.dma_gather(
                            out_ap=gpr[:, out_b0 * REC:(out_b0 + L // 128) * REC]
                            .rearrange("p (j e) -> p j e", e=REC),
                            in_ap=tables[c][:],
                            idxs_ap=ix[:, coff - c0:coff - c0 + L // 16],
                            num_idxs=L, num_idxs_reg=L,
                            elem_size=REC, queue_num=next_q())
                    for t in pair_tiles[p]:
                        so = selfoff[t]
                        nc.scalar.dma_start(
                            gpr[:, so * REC:(so + 1) * REC],
                            own_rec.rearrange("(p t) e -> p t e", p=128)[:, t, :])
                    gprs[p] = gpr

                emit_gathers(0)
                emit_gathers(1)

                stage = {}

                def stage_a(t):
                    p = t // 2
                    ns1 = int(ns1_t[t])
                    gpr = gprs[p]
                    m1t = m1p.tile([128, MAXNS1 * 128], fp8, tag="m1")
                    nc.sync.dma_start(
                        m1t[:, 0:ns1 * 128],
                        m1all[:, int(m1off[t]) * 128:(int(m1off[t]) + ns1) * 128])
                    m2t = m2p.tile([128, MAXNS1 * 128], fp8, tag="m2")
                    nc.gpsimd.dma_start(
                        m2t[:, 0:ns1 * 128],
                        m2all[:, int(m1off[t]) * 128:(int(m1off[t]) + ns1) * 128])
                    ape = pape.tile([128, MAXNS1 * 4], f32, tag="ape")
                    for j in range(ns1):
                        nc.tensor.matmul(ape[:, j * 4:(j + 1) * 4],
                                         lhsT=m2t[:, j * 128:(j + 1) * 128],
                                         rhs=r_sb[:, t * 4:(t + 1) * 4],
                                         start=True, stop=True)
                    # w = max(e^s, e^(0.2 s) * r_edge); s gathered per edge
                    gh = gpr.bitcast(bf16).rearrange("p (j x) -> p j x", x=RECH)
                    scmp = smp.tile([128, MAXNS1 * 4], bf16, tag="scmp")
                    for (j0, k, b0) in segs_t[t]:
                        nc.scalar.copy(
                            scmp[:, j0 * 4:(j0 + k) * 4].rearrange(
                                "p (j h) -> p j h", h=4),
                            gh[:, b0:b0 + k, 130:134])
                    u = smp.tile([128, MAXNS1 * 4], bf16, tag="u")
                    nc.scalar.activation(out=u[:, 0:ns1 * 4],
                                         in_=scmp[:, 0:ns1 * 4], func=AF.Exp)
                    vr = smp.tile([128, MAXNS1 * 4], bf16, tag="vr")
                    nc.scalar.activation(out=vr[:, 0:ns1 * 4],
                                         in_=scmp[:, 0:ns1 * 4], func=AF.Exp,
                                         scale=0.2)
                    nc.vector.tensor_tensor(out=vr[:, 0:ns1 * 4],
                                            in0=vr[:, 0:ns1 * 4],
                                            in1=ape[:, 0:ns1 * 4], op=OP.mult)
                    w = smp.tile([128, MAXNS1 * 4], bf16, tag="w")
                    nc.vector.tensor_tensor(out=w[:, 0:ns1 * 4],
                                            in0=u[:, 0:ns1 * 4],
                                            in1=vr[:, 0:ns1 * 4], op=OP.max)
                    stage[t] = (gpr, m1t, w, ns1)

                def stage_b(t):
                    gpr, m1t, w, ns1 = stage.pop(t)
                    agg = pagg.tile([128, 260], f32, tag="agg")
                    # pair up consecutive region blocks for one DVE op each
                    rbl = blocks_t[t]
                    j = 0
                    mulq = 0
                    while j < ns1:
                        if False and j + 1 < ns1 and rbl[j + 1] == rbl[j] + 1:
                            npair = 2
                            rhs = rhp.tile([128, 520], bf16, tag="rhs2")
                            nc.vector.scalar_tensor_tensor(
                                out=rhs[:].rearrange("p (j g h) -> p j g h",
                                                     j=2, h=4),
                                in0=gpr[:, rbl[j] * REC:(rbl[j] + 2) * REC]
                                .rearrange("p (j e) -> p j e", e=REC)[:, :, 0:260]
                                .rearrange("p j (g h) -> p j g h", h=4),
                                scalar=1.0,
                                in1=w[:, j * 4:(j + 2) * 4]
                                .rearrange("p (j o h) -> p j o h", j=2, o=1)
                                .to_broadcast([128, 2, 65, 4]),
                                op0=OP.mult, op1=OP.mult)
                        else:
                            npair = 1
                            rhs = rhp.tile([128, 260], bf16, tag="rhs")
                            nc.vector.scalar_tensor_tensor(
                                out=rhs[:].rearrange("p (g h) -> p g h", h=4),
                                in0=gpr[:, rbl[j] * REC:rbl[j] * REC + 260]
                                .rearrange("p (g h) -> p g h", h=4),
                                scalar=1.0,
                                in1=w[:, j * 4:(j + 1) * 4].rearrange(
                                    "p (o h) -> p o h", o=1)
                                .to_broadcast([128, 65, 4]),
                                op0=OP.mult, op1=OP.mult)
                        for q in range(npair):
                            nc.tensor.matmul(
                                agg[:],
                                lhsT=m1t[:, (j + q) * 128:(j + q + 1) * 128],
                                rhs=rhs[:, q * 260:(q + 1) * 260],
                                start=(j + q == 0), stop=(j + q == ns1 - 1))
                        j += npair
                    recip = smp.tile([128, 4], f32, tag="recip")
                    if t == TILES - 1:
                        # pad lanes of the last tile have dn == 0
                        dn = smp.tile([128, 4], f32, tag="dn")
                        nc.vector.tensor_scalar(out=dn[:], in0=agg[:, 256:260],
                                                scalar1=1e-6, scalar2=None,
                                                op0=OP.add)
                        nc.vector.reciprocal(recip[:], dn[:])
                    else:
                        nc.vector.reciprocal(recip[:], agg[:, 256:260])
                    tmp = smp.tile([128, 256], bf16, tag="tmp")
                    nc.vector.tensor_tensor(
                        out=tmp[:].rearrange("p (g h) -> p g h", h=4),
                        in0=agg[:, 0:256].rearrange("p (g h) -> p g h", h=4),
                        in1=recip[:].rearrange("p (o h) -> p o h", o=1)
                        .to_broadcast([128, 64, 4]),
                        op=OP.mult)
                    hacc = smp.tile([128, C], f32, tag="hacc")
                    nc.vector.tensor_reduce(
                        out=hacc[:], in_=tmp[:].rearrange("p (g h) -> p g h", h=4),
                        axis=mybir.AxisListType.X, op=OP.add)
                    hsl = h_sb[:, t * C:(t + 1) * C]
                    nc.vector.scalar_tensor_tensor(
                        out=hsl, in0=hacc[:], scalar=1.0 / H,
                        in1=resid_sb[:, t * C:(t + 1) * C],
                        op0=OP.mult, op1=OP.add)
                    sq = smp.tile([128, C], bf16, tag="sq")
                    nc.scalar.square(sq[:], hsl)
                    st = pst.tile([8, 2 * C], f32, tag="st")
                    nc.tensor.matmul(st[:, 0:C], lhsT=ohb_sb[:, t * 8:(t + 1) * 8],
                                     rhs=hsl, start=True, stop=True)
                    nc.tensor.matmul(st[:, C:2 * C],
                                     lhsT=ohb_sb[:, t * 8:(t + 1) * 8],
                                     rhs=sq[:], start=True, stop=True)
                    nc.vector.tensor_tensor(out=stats_sb[:], in0=stats_sb[:],
                                            in1=st[:], op=OP.add)

                for t in range(TILES):
                    if t % 2 == 0 and t // 2 + 2 < NPAIR:
                        emit_gathers(t // 2 + 2)
                    stage_a(t)
                    if t > 0:
                        stage_b(t - 1)
                stage_b(TILES - 1)

            # ---- phase 3: AllReduce stats, normalize, gelu, write out ----
            with tc.tile_pool(name="psum3", bufs=2, space="PSUM") as ps3:
                nc.gpsimd.dma_start(cc_in[:], stats_sb[:])
                nc.gpsimd.collective_compute(
                    "AllReduce", OP.add,
                    replica_groups=[list(range(NCORE))],
                    ins=[cc_in[:]], outs=[cc_out[:]])
                sall = smp.tile([8, 2 * C], f32, tag="sall")
                nc.sync.dma_start(sall[:], cc_out[:])
                gw = gn_sb[:, 0:C]
                gb = gn_sb[:, C:2 * C]
                gms = gn_sb[:, 2 * C:3 * C]
                gms2m = gn_sb[:, 3 * C:4 * C]
                invc = gn_sb[:, 4 * C:4 * C + 1]
                epsc = gn_sb[:, 4 * C + 1:4 * C + 2]
                mean = smp.tile([8, C], f32, tag="mean")
                nc.vector.tensor_scalar(out=mean[:], in0=sall[:, 0:C],
                                        scalar1=invc, scalar2=None, op0=OP.mult)
                eh2 = smp.tile([8, C], f32, tag="eh2")
                nc.vector.tensor_scalar(out=eh2[:], in0=sall[:, C:2 * C],
                                        scalar1=invc, scalar2=None, op0=OP.mult)
                msq = smp.tile([8, C], f32, tag="msq")
                nc.vector.tensor_tensor(out=msq[:], in0=mean[:], in1=mean[:],
                                        op=OP.mult)
                var = smp.tile([8, C], f32, tag="var")
                nc.vector.tensor_tensor(out=msq[:], in0=msq[:], in1=gms2m,
                                        op=OP.mult)
                nc.vector.tensor_tensor(out=var[:], in0=eh2[:], in1=msq[:],
                                        op=OP.subtract)
                std = smp.tile([8, C], f32, tag="std")
                nc.scalar.activation(out=std[:], in_=var[:], func=AF.Sqrt,
                                     bias=epsc)
                ab = smp.tile([8, 2 * C], f32, tag="ab")
                nc.vector.reciprocal(std[:], std[:])
                nc.vector.tensor_tensor(out=ab[:, 0:C], in0=gw, in1=std[:],
                                        op=OP.mult)
                tm = smp.tile([8, C], f32, tag="tm")
                nc.vector.tensor_tensor(out=tm[:], in0=ab[:, 0:C], in1=mean[:],
                                        op=OP.mult)
                nc.vector.tensor_tensor(out=tm[:], in0=tm[:], in1=gms,
                                        op=OP.mult)
                nc.vector.tensor_tensor(out=ab[:, C:2 * C], in0=gb, in1=tm[:],
                                        op=OP.subtract)
                abb = smp.tile([8, 2 * C], bf16, tag="abb")
                nc.vector.tensor_copy(abb[:], ab[:])

                for t in range(TILES):
                    oht = ohp.tile([8, 128], bf16, tag="oht")
                    nc.sync.dma_start(oht[:], onehotT[:, t * 128:(t + 1) * 128])
                    abpe = ps3.tile([128, 2 * C], f32, tag="abpe")
                    nc.tensor.matmul(abpe[:], lhsT=oht[:], rhs=abb[:],
                                     start=True, stop=True)
                    nrm = smp.tile([128, C], f32, tag="nrm")
                    nc.vector.tensor_tensor(out=nrm[:],
                                            in0=h_sb[:, t * C:(t + 1) * C],
                                            in1=abpe[:, 0:C], op=OP.mult)
                    nc.vector.tensor_tensor(out=nrm[:], in0=nrm[:],
                                            in1=abpe[:, C:2 * C], op=OP.add)
                    ot = otp.tile([128, C], f32, tag="ot")
                    nc.scalar.activation(out=ot[:], in_=nrm[:],
                                         func=AF.Gelu_apprx_tanh)
                    rows = 128 if t < TILES - 1 else LAST_ROWS
                    nc.sync.dma_start(out[t * 128:t * 128 + rows, :],
                                      ot[:rows, :])

    nc.compile()
    return nc


def kernel(**inputs):
    from concourse.bass_utils import run_bass_kernel_spmd

    cfg, in_maps = _host_prep(**inputs)
    nc = _build_nc(cfg)
    res = run_bass_kernel_spmd(nc, in_maps, core_ids=list(range(NCORE)))
    return np.concatenate([res.results[k]["out"] for k in range(NCORE)], axis=0)
